# revision 10
# baseline (speedup 1.0000x reference)
"""MoE-LoRA double GEMM on 8 Trainium2 NeuronCores (fp16 I/O).

Computes, for E=4 experts:  h_e = x @ A_e^T ; y_e = h_e @ B_e^T
with x:[4,2048,4096] f32, A:[4,64,4096], B:[4,4096,64] ->
y:[4,4,2048,4096] f32.

Strategy: data-parallel shard x over tokens (8192 tokens -> 1024/core),
replicate the small expert weights. All device I/O is fp16 (the extra
rounding is ~3e-4 rel err, far under the 2e-2 gate), which halves HBM
traffic vs fp32 (~46 MB/core) and doubles TensorE throughput; PSUM
accumulation stays fp32. Host prepares matmul-native layouts (x^T with
the contraction dim D leading, A/B transposed + expert-pair packed):
  GEMM1: h^T[pair] = [A_2p^T | A_2p+1^T] (stationary, experts packed on
         the M axis) x x^T tile (moving, N=512) accumulated over D.
  GEMM2: y_e tile [128 tok, 512 out] = h_e^T chunk (stationary, K=64,
         the two experts of a pair on row strips 0/64 so their matmuls
         run concurrently in the PE array) x B_e^T (moving), giving y
         in natural [token, out] layout; a full O=4096 row per 128
         tokens is staged in SBUF as fp16 and stored as one 1 MiB DMA.
GEMM1 matmuls of tile t+1 are interleaved into GEMM2 of tile t in PE
program order so the PE array keeps working while GEMM2 waits on PSUM
bank evacuation. PSUM->SBUF evacuation (with the f32->f16 cast)
alternates VectorE/ScalarE. The kernel is HBM-bandwidth bound.
"""

import os
import sys

import numpy as np

for _p in ("/opt/trn_rl_repo", "/root/.axon_site/_ro/trn_rl_repo"):
    if os.path.isdir(_p) and _p not in sys.path:
        sys.path.append(_p)

from concourse import bacc, mybir, tile
from concourse.bass_utils import run_bass_kernel_spmd

E = 4
R_E = 64
D = 4096
O = 4096
B_DIM = 4
S = 2048
T = B_DIM * S          # 8192 tokens total
NCORES = 8
TL = T // NCORES       # 1024 tokens per core
NT = 4                 # pipeline tiles per core
TT = TL // NT          # 256 tokens per tile
NCD = D // 128         # 32 contraction chunks for GEMM1
NG4 = NCD // 4         # 8 groups of 4 chunks (one x DMA each)
NGRP = TT // 128       # 4 token groups of 128 per tile

F16 = mybir.dt.float16
FP32 = mybir.dt.float32

_CACHE = {}


def _build_nc():
    nc = bacc.Bacc(None, target_bir_lowering=False, debug=False)
    xt_d = nc.declare_dram_parameter("xT", [NT, 128, NG4, 4, TT], F16, isOutput=False)
    at_d = nc.declare_dram_parameter("AT", [2, 128, NCD, 128], F16, isOutput=False)
    bt_d = nc.declare_dram_parameter("BT", [2, 128, O], F16, isOutput=False)
    y_d = nc.declare_dram_parameter("y", [E, TL, O], F16, isOutput=True)

    with tile.TileContext(nc) as tc:
        with (
            tc.tile_pool(name="ps_y", bufs=3, space="PSUM") as ps_y,
            tc.tile_pool(name="ps_ht", bufs=2, space="PSUM") as ps_ht,
            tc.tile_pool(name="atc", bufs=2) as atpool,
            tc.tile_pool(name="btc", bufs=2) as btpool,
            tc.tile_pool(name="xt", bufs=NT * NG4) as xtpool,
            tc.tile_pool(name="ht", bufs=2 * NT) as htpool,
            tc.tile_pool(name="ys", bufs=8) as yspool,
        ):
            # ---- loads (ScalarE HWDGE ring; stores go on the SyncE ring) ----
            atc = []
            for p in range(2):
                ac = atpool.tile([128, NCD, 128], F16, name=f"at{p}", tag="atc")
                nc.scalar.dma_start(out=ac[:], in_=at_d[p])
                atc.append(ac)
            xqs = []
            for tt in range(NT):
                xq = []
                for g4 in range(NG4):
                    xc = xtpool.tile([128, 4, TT], F16, name=f"xc{tt}_{g4}", tag="xtc")
                    nc.scalar.dma_start(out=xc[:], in_=xt_d[tt, :, g4])
                    xq.append(xc)
                xqs.append(xq)
                if tt == 0:
                    btc = []
                    for p in range(2):
                        bc = btpool.tile([128, O], F16, name=f"bt{p}", tag="btc")
                        nc.scalar.dma_start(out=bc[:], in_=bt_d[p])
                        btc.append(bc)

            def g1_mms(phts, tnext, c):
                for p in range(2):
                    nc.tensor.matmul(
                        phts[p][:, :TT],
                        atc[p][:, c, :],
                        xqs[tnext][c // 4][:, c % 4, :],
                        start=(c == 0),
                        stop=(c == NCD - 1),
                    )

            def h_copies(phts, tnext):
                hts = []
                for p in range(2):
                    ht = htpool.tile([128, TT], F16, name=f"ht{tnext}_{p}", tag="ht")
                    if p == 0:
                        nc.vector.tensor_copy(ht[:], phts[p][:, :TT])
                    else:
                        nc.scalar.copy(ht[:], phts[p][:, :TT])
                    hts.append(ht)
                return hts

            # GEMM1 for tile 0 stands alone; later tiles interleave into GEMM2.
            # pht tiles padded to a full 2 KiB PSUM bank ([128, 512] fp32)
            phts = [
                ps_ht.tile([128, 512], FP32, name=f"pht0_{p}", tag="pht")
                for p in range(2)
            ]
            for c in range(NCD):
                g1_mms(phts, 0, c)
            hts = h_copies(phts, 0)

            for tt in range(NT):
                nxt = tt + 1 < NT
                if nxt:
                    phts_n = [
                        ps_ht.tile([128, 512], FP32, name=f"pht{tt + 1}_{p}", tag="pht")
                        for p in range(2)
                    ]
                for p in range(2):
                    for g in range(NGRP):
                        unit = p * NGRP + g
                        ysq = [
                            yspool.tile([128, O], F16, name=f"ys{tt}_{p}_{g}_{s}", tag="ys")
                            for s in range(2)
                        ]
                        for oc2 in range(4):
                            pys = [
                                ps_y.tile([128, 1024], FP32, name=f"py{tt}_{unit}_{oc2}_{s}", tag="py")
                                for s in range(2)
                            ]
                            for half in range(2):
                                for s in range(2):
                                    r0 = 64 * s
                                    col = oc2 * 1024 + half * 512
                                    nc.tensor.matmul(
                                        pys[s][:, half * 512 : half * 512 + 512],
                                        hts[p][r0 : r0 + 64, g * 128 : (g + 1) * 128],
                                        btc[p][r0 : r0 + 64, col : col + 512],
                                        start=True,
                                        stop=True,
                                    )
                            for s in range(2):
                                dst = ysq[s][:, oc2 * 1024 : (oc2 + 1) * 1024]
                                if (oc2 + s) % 2 == 0:
                                    nc.vector.tensor_copy(dst, pys[s][:])
                                else:
                                    nc.scalar.copy(dst, pys[s][:])
                        # dense G1 block for the next tile at the group
                        # boundary: keeps GEMM2's two row-strip stationaries
                        # co-resident (LDWEIGHTS stays pipelined) instead of
                        # thrashing the weight buffers every third matmul
                        if nxt:
                            nun = NCD // (2 * NGRP)
                            for c in range(unit * nun, (unit + 1) * nun):
                                g1_mms(phts_n, tt + 1, c)
                        for s in range(2):
                            e = 2 * p + s
                            r0 = tt * TT + g * 128
                            nc.sync.dma_start(
                                out=y_d[e, r0 : r0 + 128, :], in_=ysq[s][:]
                            )
                if nxt:
                    hts = h_copies(phts_n, tt + 1)
    nc.compile()
    return nc


def _get_nc():
    if "nc" not in _CACHE:
        _CACHE["nc"] = _build_nc()
    return _CACHE["nc"]


def _prep_weights(A, B):
    A = np.asarray(A, dtype=np.float32)
    B = np.asarray(B, dtype=np.float32)
    at = np.empty((2, 128, NCD, 128), dtype=np.float16)
    bt = np.empty((2, 128, O), dtype=np.float16)
    for p in range(2):
        # stationary for GEMM1: [D, 128] with expert 2p in cols 0-63, 2p+1 in 64-127
        atp = np.concatenate([A[2 * p].T, A[2 * p + 1].T], axis=1)  # [4096, 128]
        at[p] = atp.reshape(NCD, 128, 128).transpose(1, 0, 2)
        # moving for GEMM2: [128, O] with expert 2p rows 0-63, 2p+1 rows 64-127
        bt[p] = np.concatenate([B[2 * p].T, B[2 * p + 1].T], axis=0)
    return at, bt


def kernel(x, A, B, _trace=False):
    x = np.asarray(x, dtype=np.float32).reshape(T, D)
    at, bt = _prep_weights(A, B)

    nc = _get_nc()
    in_maps = []
    for k in range(NCORES):
        xk = x[k * TL : (k + 1) * TL].astype(np.float16)  # [TL, D]
        # [tt, p, g4, j, t] = xk[tt*TT + t, (g4*4 + j)*128 + p]
        xh = np.ascontiguousarray(
            xk.reshape(NT, TT, NG4, 4, 128).transpose(0, 4, 2, 3, 1)
        )
        in_maps.append({"xT": xh, "AT": at, "BT": bt})
    res = run_bass_kernel_spmd(nc, in_maps, list(range(NCORES)), trace=_trace)
    if _trace:
        _CACHE["last_result"] = res

    y = np.empty((E, T, O), dtype=np.float16)
    for k in range(NCORES):
        y[:, k * TL : (k + 1) * TL, :] = res.results[k]["y"]
    return y.reshape(E, B_DIM, S, O).astype(np.float32)


# revision 11
# speedup vs baseline: 1.0331x; 1.0331x over previous
"""MoE-LoRA double GEMM on 8 Trainium2 NeuronCores (fp16 I/O).

Computes, for E=4 experts:  h_e = x @ A_e^T ; y_e = h_e @ B_e^T
with x:[4,2048,4096] f32, A:[4,64,4096], B:[4,4096,64] ->
y:[4,4,2048,4096] f32.

Strategy: data-parallel shard x over tokens (8192 tokens -> 1024/core),
replicate the small expert weights. All device I/O is fp16 (the extra
rounding is ~4e-4 rel err, far under the 2e-2 gate), which halves HBM
traffic vs fp32 (~46 MB/core) and doubles TensorE throughput; PSUM
accumulation stays fp32. Host prepares matmul-native layouts (x^T with
the contraction dim D leading, A/B transposed + expert-pair packed).
x is split into a small "head" section (first 256 tokens; 8 small DMAs
so GEMM1 of tile 0 finishes early and the store stream starts ~18us in)
and a "body" section (768 tokens; 8 large 768 KiB DMAs for DMA
efficiency), giving pipeline tiles of [256, 256, 512] tokens.
  GEMM1: h^T[pair] = [A_2p^T | A_2p+1^T] (stationary, experts packed on
         the M axis) x x^T tile (moving) accumulated over D.
  GEMM2: y_e tile [128 tok, 512 out] = h_e^T chunk (stationary, K=64,
         the two experts of a pair on row strips 0/64 so their matmuls
         run concurrently in the PE array) x B_e^T (moving), giving y
         in natural [token, out] layout; a full O=4096 row per 128
         tokens is staged in SBUF as fp16 and stored as one 1 MiB DMA.
GEMM1 matmuls of tile t+1 run as dense blocks at GEMM2 group boundaries
(not finer) so LDWEIGHTS stays pipelined: uniform streams keep the
foreground/background weight buffers alternating, while mixing a third
stationary between GEMM2's two row-strip stationaries serializes the
weight loads. PSUM->SBUF evacuation (with the f32->f16 cast) is split
VectorE/ScalarE per output tile. The kernel is HBM/DMA bound.
"""

import os
import sys

import numpy as np

for _p in ("/opt/trn_rl_repo", "/root/.axon_site/_ro/trn_rl_repo"):
    if os.path.isdir(_p) and _p not in sys.path:
        sys.path.append(_p)

from concourse import bacc, mybir, tile
from concourse.bass_utils import run_bass_kernel_spmd

E = 4
R_E = 64
D = 4096
O = 4096
B_DIM = 4
S = 2048
T = B_DIM * S          # 8192 tokens total
NCORES = 8
TL = T // NCORES       # 1024 tokens per core
TTS = [256, 256, 512]  # pipeline tile sizes (tokens); sum == TL
T_HEAD = TTS[0]        # x "head" section: loaded in small per-group DMAs
T_BODY = TL - T_HEAD   # x "body" section: loaded in 8 large DMAs
NT = len(TTS)
NCD = D // 128         # 32 contraction chunks for GEMM1
NG4 = NCD // 4         # 8 groups of 4 chunks (one x DMA each)

F16 = mybir.dt.float16
FP32 = mybir.dt.float32

_CACHE = {}


def _build_nc():
    nc = bacc.Bacc(None, target_bir_lowering=False, debug=False)
    xa_d = nc.declare_dram_parameter("xA", [128, NG4, 4, T_HEAD], F16, isOutput=False)
    xb_d = nc.declare_dram_parameter("xB", [128, NG4, 4, T_BODY], F16, isOutput=False)
    at_d = nc.declare_dram_parameter("AT", [2, 128, NCD, 128], F16, isOutput=False)
    bt_d = nc.declare_dram_parameter("BT", [2, 128, O], F16, isOutput=False)
    y_d = nc.declare_dram_parameter("y", [E, TL, O], F16, isOutput=True)

    # token offset of each tile within the body section (tile 0 is the head)
    body_off = [None]
    off = 0
    for tt in range(1, NT):
        body_off.append(off)
        off += TTS[tt]

    with tile.TileContext(nc) as tc:
        with (
            tc.tile_pool(name="ps_y", bufs=3, space="PSUM") as ps_y,
            tc.tile_pool(name="ps_ht", bufs=2, space="PSUM") as ps_ht,
            tc.tile_pool(name="atc", bufs=2) as atpool,
            tc.tile_pool(name="btc", bufs=2) as btpool,
            tc.tile_pool(name="xa", bufs=NG4) as xapool,
            tc.tile_pool(name="xb", bufs=NG4) as xbpool,
            tc.tile_pool(name="ht", bufs=2 * NT) as htpool,
            tc.tile_pool(name="ys", bufs=8) as yspool,
        ):
            # ---- loads (ScalarE HWDGE ring; stores go on the SyncE ring) ----
            atc = []
            for p in range(2):
                ac = atpool.tile([128, NCD, 128], F16, name=f"at{p}", tag="atc")
                nc.scalar.dma_start(out=ac[:], in_=at_d[p])
                atc.append(ac)
            xaq = []
            for g4 in range(NG4):
                xc = xapool.tile([128, 4, T_HEAD], F16, name=f"xa{g4}", tag="xac")
                nc.scalar.dma_start(out=xc[:], in_=xa_d[:, g4])
                xaq.append(xc)
            btc = []
            for p in range(2):
                bc = btpool.tile([128, O], F16, name=f"bt{p}", tag="btc")
                nc.scalar.dma_start(out=bc[:], in_=bt_d[p])
                btc.append(bc)
            xbq = []
            for g4 in range(NG4):
                xc = xbpool.tile([128, 4, T_BODY], F16, name=f"xb{g4}", tag="xbc")
                nc.scalar.dma_start(out=xc[:], in_=xb_d[:, g4])
                xbq.append(xc)

            def g1_rhs(tnext, c):
                if tnext == 0:
                    return xaq[c // 4][:, c % 4, :]
                o = body_off[tnext]
                return xbq[c // 4][:, c % 4, o : o + TTS[tnext]]

            def g1_mms(phts, tnext, c):
                for p in range(2):
                    nc.tensor.matmul(
                        phts[p][:, : TTS[tnext]],
                        atc[p][:, c, :],
                        g1_rhs(tnext, c),
                        start=(c == 0),
                        stop=(c == NCD - 1),
                    )

            def h_copies(phts, tnext):
                hts = []
                for p in range(2):
                    ht = htpool.tile([128, 512], F16, name=f"ht{tnext}_{p}", tag="ht")
                    if p == 0:
                        nc.vector.tensor_copy(ht[:, : TTS[tnext]], phts[p][:, : TTS[tnext]])
                    else:
                        nc.scalar.copy(ht[:, : TTS[tnext]], phts[p][:, : TTS[tnext]])
                    hts.append(ht)
                return hts

            # GEMM1 for tile 0 stands alone; later tiles interleave into GEMM2.
            phts = [
                ps_ht.tile([128, 512], FP32, name=f"pht0_{p}", tag="pht")
                for p in range(2)
            ]
            for c in range(NCD):
                g1_mms(phts, 0, c)
            hts = h_copies(phts, 0)

            tok0 = 0
            for tt in range(NT):
                ngrp = TTS[tt] // 128
                nunits = 2 * ngrp
                nxt = tt + 1 < NT
                if nxt:
                    phts_n = [
                        ps_ht.tile([128, 512], FP32, name=f"pht{tt + 1}_{p}", tag="pht")
                        for p in range(2)
                    ]
                    cpu = NCD // nunits  # G1 chunks emitted per GEMM2 group
                for p in range(2):
                    for g in range(ngrp):
                        unit = p * ngrp + g
                        ysq = [
                            yspool.tile([128, O], F16, name=f"ys{tt}_{p}_{g}_{s}", tag="ys")
                            for s in range(2)
                        ]
                        for oc2 in range(4):
                            pys = [
                                ps_y.tile([128, 1024], FP32, name=f"py{tt}_{unit}_{oc2}_{s}", tag="py")
                                for s in range(2)
                            ]
                            for half in range(2):
                                for s in range(2):
                                    r0 = 64 * s
                                    col = oc2 * 1024 + half * 512
                                    nc.tensor.matmul(
                                        pys[s][:, half * 512 : half * 512 + 512],
                                        hts[p][r0 : r0 + 64, g * 128 : (g + 1) * 128],
                                        btc[p][r0 : r0 + 64, col : col + 512],
                                        start=True,
                                        stop=True,
                                    )
                            for s in range(2):
                                dst = ysq[s][:, oc2 * 1024 : (oc2 + 1) * 1024]
                                if (oc2 + s) % 2 == 0:
                                    nc.vector.tensor_copy(dst, pys[s][:])
                                else:
                                    nc.scalar.copy(dst, pys[s][:])
                        # dense G1 block for the next tile at the group
                        # boundary: keeps GEMM2's two row-strip stationaries
                        # co-resident (LDWEIGHTS stays pipelined) instead of
                        # thrashing the weight buffers every third matmul
                        if nxt:
                            for c in range(unit * cpu, (unit + 1) * cpu):
                                g1_mms(phts_n, tt + 1, c)
                        for s in range(2):
                            e = 2 * p + s
                            r0 = tok0 + g * 128
                            nc.sync.dma_start(
                                out=y_d[e, r0 : r0 + 128, :], in_=ysq[s][:]
                            )
                if nxt:
                    hts = h_copies(phts_n, tt + 1)
                tok0 += TTS[tt]
    nc.compile()
    return nc


def _get_nc():
    if "nc" not in _CACHE:
        _CACHE["nc"] = _build_nc()
    return _CACHE["nc"]


def _prep_weights(A, B):
    A = np.asarray(A, dtype=np.float32)
    B = np.asarray(B, dtype=np.float32)
    at = np.empty((2, 128, NCD, 128), dtype=np.float16)
    bt = np.empty((2, 128, O), dtype=np.float16)
    for p in range(2):
        # stationary for GEMM1: [D, 128] with expert 2p in cols 0-63, 2p+1 in 64-127
        atp = np.concatenate([A[2 * p].T, A[2 * p + 1].T], axis=1)  # [4096, 128]
        at[p] = atp.reshape(NCD, 128, 128).transpose(1, 0, 2)
        # moving for GEMM2: [128, O] with expert 2p rows 0-63, 2p+1 rows 64-127
        bt[p] = np.concatenate([B[2 * p].T, B[2 * p + 1].T], axis=0)
    return at, bt


def _pack_x(xk, n_tok):
    # [p, g4, j, t] = xk[t, (g4*4 + j)*128 + p]
    return np.ascontiguousarray(
        xk.reshape(n_tok, NG4, 4, 128).transpose(3, 1, 2, 0)
    )


def kernel(x, A, B, _trace=False):
    x = np.asarray(x, dtype=np.float32).reshape(T, D)
    at, bt = _prep_weights(A, B)

    nc = _get_nc()
    in_maps = []
    for k in range(NCORES):
        xk = x[k * TL : (k + 1) * TL].astype(np.float16)  # [TL, D]
        in_maps.append(
            {
                "xA": _pack_x(xk[:T_HEAD], T_HEAD),
                "xB": _pack_x(xk[T_HEAD:], T_BODY),
                "AT": at,
                "BT": bt,
            }
        )
    res = run_bass_kernel_spmd(nc, in_maps, list(range(NCORES)), trace=_trace)
    if _trace:
        _CACHE["last_result"] = res

    y = np.empty((E, T, O), dtype=np.float16)
    for k in range(NCORES):
        y[:, k * TL : (k + 1) * TL, :] = res.results[k]["y"]
    return y.reshape(E, B_DIM, S, O).astype(np.float32)


# revision 15
# speedup vs baseline: 1.0712x; 1.0369x over previous
"""MoE-LoRA double GEMM on 8 Trainium2 NeuronCores (fp16 I/O).

Computes, for E=4 experts:  h_e = x @ A_e^T ; y_e = h_e @ B_e^T
with x:[4,2048,4096] f32, A:[4,64,4096], B:[4,4096,64] ->
y:[4,4,2048,4096] f32.

Strategy: data-parallel shard x over tokens (8192 tokens -> 1024/core),
replicate the small expert weights. All device I/O is fp16 (the extra
rounding is ~4e-4 rel err, far under the 2e-2 gate), which halves HBM
traffic vs fp32 (~46 MB/core) and doubles TensorE throughput; PSUM
accumulation stays fp32. Host prepares matmul-native layouts (x^T with
the contraction dim D leading, A/B transposed + expert-pair packed).
x is split into a small "head" section (first 256 tokens; 8 small DMAs
so GEMM1 of tile 0 finishes early and the store stream starts ~18us in)
and a "body" section (768 tokens; 8 large 768 KiB DMAs for DMA
efficiency), giving pipeline tiles of [256, 256, 512] tokens.
  GEMM1: h^T[pair] = [A_2p^T | A_2p+1^T] (stationary, experts packed on
         the M axis) x x^T tile (moving) accumulated over D.
  GEMM2: y_e tile [128 tok, 512 out] = h_e^T chunk (stationary, K=64,
         the two experts of a pair on row strips 0/64 so their matmuls
         run concurrently in the PE array) x B_e^T (moving), giving y
         in natural [token, out] layout; a full O=4096 row per 128
         tokens is staged in SBUF as fp16 and stored as one 1 MiB DMA.
GEMM1 matmuls of tile t+1 run as dense blocks at GEMM2 group boundaries
(not finer) so LDWEIGHTS stays pipelined: uniform streams keep the
foreground/background weight buffers alternating, while mixing a third
stationary between GEMM2's two row-strip stationaries serializes the
weight loads. PSUM->SBUF evacuation (with the f32->f16 cast) is split
VectorE/ScalarE per output tile. The kernel is HBM/DMA bound.
"""

import os
import sys

import numpy as np

for _p in ("/opt/trn_rl_repo", "/root/.axon_site/_ro/trn_rl_repo"):
    if os.path.isdir(_p) and _p not in sys.path:
        sys.path.append(_p)

from concourse import bacc, mybir, tile
from concourse.bass_utils import run_bass_kernel_spmd

E = 4
R_E = 64
D = 4096
O = 4096
B_DIM = 4
S = 2048
T = B_DIM * S          # 8192 tokens total
NCORES = 8
TL = T // NCORES       # 1024 tokens per core
TTS = [256, 256, 512]  # pipeline tile sizes (tokens); sum == TL
T_HEAD = TTS[0]        # x "head" section: loaded in small per-group DMAs
T_BODY = TL - T_HEAD   # x "body" section: loaded in 8 large DMAs
NT = len(TTS)
NCD = D // 128         # 32 contraction chunks for GEMM1
NG4 = NCD // 4         # 8 groups of 4 chunks (one x DMA each)

F16 = mybir.dt.float16
FP32 = mybir.dt.float32

_CACHE = {}


def _build_nc():
    nc = bacc.Bacc(None, target_bir_lowering=False, debug=False)
    x_d = [
        nc.declare_dram_parameter(f"x{tt}", [128, NG4, 4, TTS[tt]], F16, isOutput=False)
        for tt in range(NT)
    ]
    at_d = nc.declare_dram_parameter("AT", [2, 128, NCD, 128], F16, isOutput=False)
    bt_d = nc.declare_dram_parameter("BT", [2, 128, O], F16, isOutput=False)
    y_d = nc.declare_dram_parameter("y", [E, TL, O], F16, isOutput=True)

    with tile.TileContext(nc) as tc:
        with (
            tc.tile_pool(name="ps_y", bufs=3, space="PSUM") as ps_y,
            tc.tile_pool(name="ps_ht", bufs=2, space="PSUM") as ps_ht,
            tc.tile_pool(name="atc", bufs=2) as atpool,
            tc.tile_pool(name="btc", bufs=2) as btpool,
            tc.tile_pool(name="xt", bufs=NG4) as xtpool,
            tc.tile_pool(name="ht", bufs=2 * NT) as htpool,
            tc.tile_pool(name="ys", bufs=8) as yspool,
        ):
            # ---- loads (ScalarE HWDGE ring; stores go on the SyncE ring) ----
            # order: A, x tile0, B, x tile1, x tile2 ... so each tile's x
            # lands well before the GEMM2 window it is interleaved into
            atc = []
            for p in range(2):
                ac = atpool.tile([128, NCD, 128], F16, name=f"at{p}", tag="atc")
                nc.scalar.dma_start(out=ac[:], in_=at_d[p])
                atc.append(ac)
            xqs = [None] * NT
            btc = []
            for tt in range(NT):
                xq = []
                for g4 in range(NG4):
                    xc = xtpool.tile(
                        [128, 4, TTS[tt]], F16, name=f"x{tt}_{g4}", tag=f"xc{tt}"
                    )
                    nc.scalar.dma_start(out=xc[:], in_=x_d[tt][:, g4])
                    xq.append(xc)
                xqs[tt] = xq
                if tt == 0:
                    for p in range(2):
                        bc = btpool.tile([128, O], F16, name=f"bt{p}", tag="btc")
                        nc.scalar.dma_start(out=bc[:], in_=bt_d[p])
                        btc.append(bc)

            def g1_rhs(tnext, c):
                return xqs[tnext][c // 4][:, c % 4, :]

            def g1_mms(phts, tnext, c):
                for p in range(2):
                    nc.tensor.matmul(
                        phts[p][:, : TTS[tnext]],
                        atc[p][:, c, :],
                        g1_rhs(tnext, c),
                        start=(c == 0),
                        stop=(c == NCD - 1),
                    )

            def h_copies(phts, tnext):
                hts = []
                for p in range(2):
                    ht = htpool.tile([128, 512], F16, name=f"ht{tnext}_{p}", tag="ht")
                    if p == 0:
                        nc.vector.tensor_copy(ht[:, : TTS[tnext]], phts[p][:, : TTS[tnext]])
                    else:
                        nc.scalar.copy(ht[:, : TTS[tnext]], phts[p][:, : TTS[tnext]])
                    hts.append(ht)
                return hts

            # GEMM1 for tile 0 stands alone; later tiles interleave into GEMM2.
            phts = [
                ps_ht.tile([128, 512], FP32, name=f"pht0_{p}", tag="pht")
                for p in range(2)
            ]
            for c in range(NCD):
                g1_mms(phts, 0, c)
            hts = h_copies(phts, 0)

            tok0 = 0
            for tt in range(NT):
                ngrp = TTS[tt] // 128
                nunits = 2 * ngrp
                nxt = tt + 1 < NT
                if nxt:
                    phts_n = [
                        ps_ht.tile([128, 512], FP32, name=f"pht{tt + 1}_{p}", tag="pht")
                        for p in range(2)
                    ]
                    cpu = NCD // nunits  # G1 chunks emitted per GEMM2 group
                for p in range(2):
                    for g in range(ngrp):
                        unit = p * ngrp + g
                        ysq = [
                            yspool.tile([128, O], F16, name=f"ys{tt}_{p}_{g}_{s}", tag="ys")
                            for s in range(2)
                        ]
                        for oc2 in range(4):
                            pys = [
                                ps_y.tile([128, 1024], FP32, name=f"py{tt}_{unit}_{oc2}_{s}", tag="py")
                                for s in range(2)
                            ]
                            for half in range(2):
                                for s in range(2):
                                    r0 = 64 * s
                                    col = oc2 * 1024 + half * 512
                                    nc.tensor.matmul(
                                        pys[s][:, half * 512 : half * 512 + 512],
                                        hts[p][r0 : r0 + 64, g * 128 : (g + 1) * 128],
                                        btc[p][r0 : r0 + 64, col : col + 512],
                                        start=True,
                                        stop=True,
                                    )
                            for s in range(2):
                                dst = ysq[s][:, oc2 * 1024 : (oc2 + 1) * 1024]
                                if (oc2 + s) % 2 == 0:
                                    nc.vector.tensor_copy(dst, pys[s][:])
                                else:
                                    nc.scalar.copy(dst, pys[s][:])
                        # dense G1 block for the next tile at the group
                        # boundary: keeps GEMM2's two row-strip stationaries
                        # co-resident (LDWEIGHTS stays pipelined) instead of
                        # thrashing the weight buffers every third matmul
                        if nxt:
                            for c in range(unit * cpu, (unit + 1) * cpu):
                                g1_mms(phts_n, tt + 1, c)
                        for s in range(2):
                            e = 2 * p + s
                            r0 = tok0 + g * 128
                            nc.sync.dma_start(
                                out=y_d[e, r0 : r0 + 128, :], in_=ysq[s][:]
                            )
                if nxt:
                    hts = h_copies(phts_n, tt + 1)
                tok0 += TTS[tt]
    nc.compile()
    return nc


def _get_nc():
    if "nc" not in _CACHE:
        _CACHE["nc"] = _build_nc()
    return _CACHE["nc"]


def _prep_weights(A, B):
    A = np.asarray(A, dtype=np.float32)
    B = np.asarray(B, dtype=np.float32)
    at = np.empty((2, 128, NCD, 128), dtype=np.float16)
    bt = np.empty((2, 128, O), dtype=np.float16)
    for p in range(2):
        # stationary for GEMM1: [D, 128] with expert 2p in cols 0-63, 2p+1 in 64-127
        atp = np.concatenate([A[2 * p].T, A[2 * p + 1].T], axis=1)  # [4096, 128]
        at[p] = atp.reshape(NCD, 128, 128).transpose(1, 0, 2)
        # moving for GEMM2: [128, O] with expert 2p rows 0-63, 2p+1 rows 64-127
        bt[p] = np.concatenate([B[2 * p].T, B[2 * p + 1].T], axis=0)
    return at, bt


def _pack_x(xk, n_tok):
    # [p, g4, j, t] = xk[t, (g4*4 + j)*128 + p]
    return np.ascontiguousarray(
        xk.reshape(n_tok, NG4, 4, 128).transpose(3, 1, 2, 0)
    )


def kernel(x, A, B, _trace=False):
    x = np.asarray(x, dtype=np.float32).reshape(T, D)
    at, bt = _prep_weights(A, B)

    nc = _get_nc()
    in_maps = []
    for k in range(NCORES):
        xk = x[k * TL : (k + 1) * TL].astype(np.float16)  # [TL, D]
        m = {"AT": at, "BT": bt}
        t0 = 0
        for tt in range(NT):
            m[f"x{tt}"] = _pack_x(xk[t0 : t0 + TTS[tt]], TTS[tt])
            t0 += TTS[tt]
        in_maps.append(m)
    res = run_bass_kernel_spmd(nc, in_maps, list(range(NCORES)), trace=_trace)
    if _trace:
        _CACHE["last_result"] = res

    y = np.empty((E, T, O), dtype=np.float16)
    for k in range(NCORES):
        y[:, k * TL : (k + 1) * TL, :] = res.results[k]["y"]
    return y.reshape(E, B_DIM, S, O).astype(np.float32)
